# revision 2
# baseline (speedup 1.0000x reference)
"""Tensor-sketch kernel v5: fused scatter->FFT stage 1 with x/y packed into
one contraction, complex-stacked single-matmul stage 2 / inverse stage 2,
r-first batched inverse stage 1.  All matmuls bf16 with f32 accumulate.

Per-core einsums (R=512):
  Tvr/Tvi = 'gnk,gkf->gnf'  xyg[64,R,2KP] @ Cv*[64,2KP,128]   (2 x 1.07 GMAC)
  V       = 'nfk,kl->nfl'   tv[R,128,128] @ Kf[128,128]       (1.07 GMAC)
  U       = 'nfk,kl->nfl'   q[R,128,128]  @ Kv[128,128]       (1.07 GMAC)
  oi      = 'rnk,rkb->rnb'  Ustk[64,R,256] @ Astk[64,256,128] (1.07 GMAC)
"""
import numpy as np
import ml_dtypes
import jax
import jax.numpy as jnp
from jax.sharding import Mesh, PartitionSpec as P
from jax.experimental.shard_map import shard_map
from functools import partial

N = 4096
D = 4096
M = 8192
M1 = 128
M2 = 64
NCORES = 8
BF = ml_dtypes.bfloat16

_cache = {}


def _stationaries():
    if "Kf" not in _cache:
        W = np.exp(-2j * np.pi / M)
        f1 = np.arange(M1)
        b1 = np.arange(M1)
        b2 = np.arange(M2)
        W64 = np.exp(-2j * np.pi * np.outer(b2, b2) / M2)
        Kr, Ki = W64.real, W64.imag
        Kf = np.zeros((128, 128), np.float32)
        Kf[0:64, 0:64] = Kr;  Kf[64:128, 0:64] = -Ki
        Kf[0:64, 64:128] = Ki; Kf[64:128, 64:128] = Kr
        Kv = np.zeros((128, 128), np.float32)
        Kv[0:64, 0:64] = Kr;  Kv[64:128, 0:64] = Ki
        Kv[0:64, 64:128] = -Ki; Kv[64:128, 64:128] = Kr
        A = W ** (f1[None, :, None] * (M2 * b1[:, None, None] + b2[None, None, :]))
        Astk = np.zeros((M2, 2 * M1, M1), np.float32)
        Astk[:, 0:M1, :] = np.transpose(A.real, (2, 1, 0))
        Astk[:, M1:2 * M1, :] = -np.transpose(A.imag, (2, 1, 0))
        _cache["Kf"] = Kf.astype(BF)
        _cache["Kv"] = Kv.astype(BF)
        _cache["Astk"] = Astk.astype(BF)
    return _cache


def _grouping(idx, sign, KP):
    g = idx % M2
    order = np.argsort(g, kind="stable")
    cnt = np.bincount(g, minlength=M2)
    off = np.concatenate([[0], np.cumsum(cnt)])
    slot = np.arange(D) - off[g[order]]
    ph = np.exp((-2j * np.pi / M) * idx[order, None] * np.arange(M1)[None, :])
    ph *= sign[order, None]
    Cr = np.zeros((M2, KP, M1), np.float32)
    Ci = np.zeros((M2, KP, M1), np.float32)
    Cr[g[order], slot, :] = ph.real
    Ci[g[order], slot, :] = ph.imag
    return order, cnt, off, Cr, Ci


def _pack(v, order, cnt, off, KP):
    vg = np.zeros((M2, v.shape[0], KP), BF)
    for grp in range(M2):
        cols = order[off[grp]:off[grp + 1]]
        vg[grp, :, :cnt[grp]] = v[:, cols]
    return vg


def _core_fn(xyg, Cvr, Cvi, Kf, Kv, Astk):
    bf = jnp.bfloat16
    mm = partial(jnp.einsum, preferred_element_type=jnp.float32)
    Tvr = mm('gnk,gkf->gnf', xyg, Cvr)
    Tvi = mm('gnk,gkf->gnf', xyg, Cvi)
    tv = jnp.transpose(
        jnp.concatenate([Tvr.astype(bf), Tvi.astype(bf)], axis=0), (1, 2, 0))
    V = mm('nfk,kl->nfl', tv, Kf)            # [n,128,(xr|xi)]
    xr, xi = V[:, :, :64], V[:, :, 64:]
    qr = (xr + xi) * (xr - xi)
    qi = 2.0 * xr * xi
    q = jnp.concatenate([qr, qi], axis=2).astype(bf)
    U = mm('nfk,kl->nfl', q, Kv)             # [n,128,(ur|ui)]
    Ut = jnp.transpose(U.astype(bf), (2, 0, 1))
    Ustk = jnp.concatenate([Ut[64:128], Ut[0:64]], axis=2)   # [64,n,(ui_f|ur_f)]
    oi = mm('rnk,rkb->rnb', Ustk, Astk)
    out = jnp.transpose(oi, (1, 2, 0)).reshape(oi.shape[1], M) * (0.5 / M)
    return out.astype(jnp.float32)


def _build(R, KP):
    devices = jax.devices()[:NCORES]
    mesh = Mesh(np.asarray(devices), ("core",))
    fn = shard_map(
        _core_fn, mesh=mesh,
        in_specs=(P(None, "core"),) + (P(),) * 5,
        out_specs=P("core"))
    return mesh, jax.jit(fn)


def _prepare(x, y, sign1, indx1, sign2, indx2):
    st = _stationaries()
    indx1 = np.asarray(indx1).astype(np.int64)
    indx2 = np.asarray(indx2).astype(np.int64)
    maxc = max(np.bincount(indx1 % M2, minlength=M2).max(),
               np.bincount(indx2 % M2, minlength=M2).max())
    KP = max(128, int(np.ceil(maxc / 128)) * 128)
    o1, c1, off1, C1r, C1i = _grouping(indx1, np.asarray(sign1, np.float64), KP)
    o2, c2, off2, C2r, C2i = _grouping(indx2, np.asarray(sign2, np.float64), KP)
    xg = _pack(np.asarray(x, np.float32), o1, c1, off1, KP)
    yg = _pack(np.asarray(y, np.float32), o2, c2, off2, KP)
    xyg = np.concatenate([xg, yg], axis=2)                     # [64, N, 2KP]
    Cvr = np.concatenate([C1r, -C2i], axis=1).astype(BF)       # [64, 2KP, 128]
    Cvi = np.concatenate([C1i, C2r], axis=1).astype(BF)
    args = (xyg, Cvr, Cvi, st["Kf"], st["Kv"], st["Astk"])
    return args, N // NCORES, KP


def kernel(x, y, sign1, indx1, sign2, indx2):
    args, R, KP = _prepare(x, y, sign1, indx1, sign2, indx2)
    key = ("jfn5", R, KP)
    if key not in _cache:
        _cache[key] = _build(R, KP)
    mesh, jfn = _cache[key]
    out = jfn(*args)
    return np.asarray(out, np.float32)


if __name__ == "__main__":
    rng = np.random.default_rng(0)
    x = rng.standard_normal((N, D)).astype(np.float32)
    y = rng.standard_normal((N, D)).astype(np.float32)
    s1 = (rng.integers(0, 2, D) * 2 - 1).astype(np.float32)
    s2 = (rng.integers(0, 2, D) * 2 - 1).astype(np.float32)
    i1 = rng.integers(0, M, D).astype(np.int32)
    i2 = rng.integers(0, M, D).astype(np.int32)
    o = kernel(x, y, s1, i1, s2, i2)
    print("kernel5 ok", o.shape, o.dtype, float(np.abs(o).max()))
